# revision 1
# baseline (speedup 1.0000x reference)
"""HQLinear (VQ codebook) Trainium2 kernel.

Computes: out = einsum('bsi,oi->bso', x, codebook[indices].reshape(O, I) * scales)
on 8 NeuronCores, sharded over out_features (512 rows per core).

Per-core pipeline:
  - codebook cast f32->bf16 on device into a 256B-row-stride padded DRAM
    tensor (dma_gather's source stride must be a multiple of 256B).
  - per 128-out-row tile: DMA-gather the tile's 1376*128 codebook vectors
    (16B each) into SBUF staging [128 o, 11008 i] bf16, then PE-transpose
    pair-packed (2 bf16 viewed as one f32 lane) into a resident SBUF wT
    (i on partitions), 11 MB bf16.
  - x streamed f32->bf16 via SWDGE cast DMA per 128-token tile,
    PE-transposed pair-packed, then 86 bf16 matmuls (N=512) accumulate
    x_tile @ w_shard.T into PSUM.
  - epilogue: multiply by scales (free-dim tile), DMA out f32.

Pair packing: an f32 lane at pair index f holds bf16 values for i = 2f,
2f+1; matmul (icp, h) contracts partitions p <-> i = 256*icp + 2p + h on
both operands via stride-2 bf16 views.
"""

from contextlib import ExitStack

import numpy as np

import concourse.ap_utils as ap_utils
import concourse.bass as bass
import concourse.tile as tile
from concourse import bacc, mybir
from concourse.bass import ts, ds, exact_div
from concourse.masks import make_identity
import concourse.bass_utils as bass_utils

F32 = mybir.dt.float32
BF16 = mybir.dt.bfloat16
I16 = mybir.dt.int16
P = 128

N_CORES = 8
OUT_F = 4096
IN_F = 11008
VDIM = 8
N_CODES = 32768
BATCH, SEQ = 2, 2048
T = BATCH * SEQ            # 4096 tokens
OSH = OUT_F // N_CORES     # 512 out rows per core
NJ = IN_F // VDIM          # 1376 index columns per out row
JC = 16                    # gather chunks per 128-row o-tile
NJC = NJ // JC             # 86 j-columns per gather (11008 idx <= HW limit)


def _dma_gather_small(gp, out_ap, in_ap, idxs_ap, num_idxs, elem_size, elem_step):
    """dma_gather with small elements (16B); source stride still 256B-aligned.

    Vector g comes from in_[list[g], :elem_size] (row stride elem_step) and
    lands at out[g%128, g//128, :]. Index list int16, wrapped: idxs[c, s] =
    list[s*16 + c] for c in 0..15, replicated across the 8 16-row groups.
    """
    assert idxs_ap.dtype == I16
    assert in_ap.dtype == out_ap.dtype
    assert in_ap.space == bass.MemorySpace.DRAM
    assert idxs_ap.space == bass.MemorySpace.SBUF
    assert out_ap.space == bass.MemorySpace.SBUF
    assert ap_utils.ap_is_contiguous(in_ap.ap[1:])
    assert ap_utils.ap_is_contiguous(out_ap.ap[1:])
    assert ap_utils.ap_is_contiguous(idxs_ap.ap[1:])
    assert in_ap.ap[-1][1] == elem_size
    assert out_ap.ap[-1][1] == elem_size
    assert in_ap.ap[0][0] == elem_step
    stride_bytes_256 = exact_div(elem_step * mybir.dt.size(in_ap.dtype), 256)
    assert 0 < stride_bytes_256 < 256
    _in_ap = gp.lower_ap_dma(in_ap, for_custom_bir_dma=True)
    _idxs_ap = gp.lower_ap(idxs_ap)
    _out_ap = gp.lower_ap(out_ap)
    return gp.add_instruction(
        mybir.InstDMAGatherAnt(
            name=gp.bass.get_next_instruction_name(),
            ins=[*_in_ap, _idxs_ap, gp.lower_val_access(gp.to_reg(num_idxs))],
            outs=[_out_ap],
            transpose=False,
            num_idxs=num_idxs,
            elem_size=elem_size,
            stride_bytes_256=stride_bytes_256,
            gen_mode=0,
            single_packet=False,
            queue_num=0,
            sbuf_tokens_per_rank=0,
            sbuf_free_dim_per_rank=0,
            sbuf_free_dim_pad_per_rank=0,
            sbuf_byte_offset=0,
        )
    )


def _emit_mms(nc, po, ent, wTb5, ICP):
    xts, g0, glen = ent
    xtsb = xts[:].bitcast(BF16)  # free: 2*(q*128 + t) + h
    for q in range(glen):
        icp = g0 + q
        for h in range(2):
            # lhsT: [128 (i=256*icp+2p+h), 128 t]
            lhsT = xtsb[:, q * 256 + h: (q + 1) * 256: 2]
            # rhs: [128 (same i map), OSH o]
            rhs = wTb5[:, icp, :, :, h]
            nc.tensor.matmul(out=po[:], lhsT=lhsT, rhs=rhs,
                             start=(icp == 0 and h == 0),
                             stop=(icp == ICP - 1 and h == 1))


def build():
    """Build and compile the per-core kernel. Returns the Bacc instance."""
    ICP = IN_F // 256          # 43 pair chunks (256 i-values each)
    O_TILES = OSH // P         # 4
    T_TILES = T // P           # 32
    GRP = 8                    # icp per transpose/copy group (2 PSUM banks)
    groups = [(g, min(GRP, ICP - g)) for g in range(0, ICP, GRP)]
    XH = [(0, (ICP + 1) // 2), ((ICP + 1) // 2, ICP)]  # x row-block halves

    nc = bacc.Bacc("TRN2", target_bir_lowering=False, debug=False,
                   enable_asserts=False, num_devices=1)

    x = nc.dram_tensor("x", [T, IN_F], F32, kind="ExternalInput").ap()
    cb = nc.dram_tensor("cb", [N_CODES, VDIM], F32, kind="ExternalInput").ap()
    idx16 = nc.dram_tensor("idx16", [O_TILES * JC, P, NJC * VDIM], I16,
                           kind="ExternalInput").ap()
    scales = nc.dram_tensor("scales", [1, OSH], F32, kind="ExternalInput").ap()
    out = nc.dram_tensor("out", [T, OSH], F32, kind="ExternalOutput").ap()
    cb_pad = nc.dram_tensor("cb_pad", [N_CODES, 128], BF16, kind="Internal").ap()

    with tile.TileContext(nc) as tc, ExitStack() as ctx:
        const_pool = ctx.enter_context(tc.tile_pool(name="const", bufs=1))
        wt_pool = ctx.enter_context(tc.tile_pool(name="wt", bufs=1))

        identity = const_pool.tile([P, P], F32)
        make_identity(nc, identity[:])

        scales_t = const_pool.tile([P, OSH], F32)
        nc.sync.dma_start(scales_t[:], scales[:].to_broadcast([P, OSH]))

        # --- codebook cast f32 -> bf16 into padded 256B-stride rows ---
        cb_flat = cb.rearrange("n v -> (n v)").rearrange("(p f) -> p f", p=P)
        NC128 = N_CODES // P
        cb_pad3 = cb_pad.rearrange("(p r) c -> p r c", p=P)[:, :, :VDIM]
        with tc.tile_pool(name="cbc", bufs=1) as cbc_pool:
            cbt = cbc_pool.tile([P, NC128 * VDIM], F32)
            cbt16 = cbc_pool.tile([P, NC128 * VDIM], BF16)
            nc.sync.dma_start(cbt[:], cb_flat)
            nc.vector.tensor_copy(cbt16[:], cbt[:])
            nc.sync.dma_start(
                cb_pad3, cbt16[:].rearrange("p (r c) -> p r c", c=VDIM))

        # --- build resident wT (pair-packed, f32-typed) ---
        # f32-lane column layout: icp * OSH + ot*128 + o
        wT = wt_pool.tile([P, ICP * OSH], F32)
        wT3 = wT[:].rearrange("p (i b) -> p i b", b=OSH)

        with tc.tile_pool(name="wstage", bufs=1) as wst_pool, \
             tc.tile_pool(name="idxp", bufs=2) as idx_pool, \
             tc.tile_pool(name="bpsum", bufs=2, space="PSUM") as bpsum_pool:
            for ot in range(O_TILES):
                wst = wst_pool.tile([P, IN_F], BF16)
                # gather: wst[p, 8j:8j+8] = bf16(cb[idx[ot*128+p, j], :])
                for jc in range(JC):
                    idx_t = idx_pool.tile([P, NJC * VDIM], I16, tag="idx")
                    nc.sync.dma_start(idx_t[:], idx16[ot * JC + jc, :, :])
                    _dma_gather_small(
                        nc.gpsimd,
                        out_ap=wst[:, jc * NJC * VDIM:(jc + 1) * NJC * VDIM]
                            .rearrange("p (n e) -> p n e", e=VDIM),
                        in_ap=cb_pad[:, :VDIM],
                        idxs_ap=idx_t[:],
                        num_idxs=NJC * P,
                        elem_size=VDIM,
                        elem_step=128,
                    )
                wstv = wst[:].bitcast(F32)  # [P, IN/2] pair lanes
                for g0, glen in groups:
                    tp = bpsum_pool.tile([P, GRP * P], F32, tag="bp")
                    for q in range(glen):
                        nc.tensor.transpose(
                            out=tp[:, ts(q, P)],
                            in_=wstv[:, ts(g0 + q, P)],
                            identity=identity[:],
                        )
                    src = tp[:, :glen * P].rearrange("p (i b) -> p i b", b=P)
                    dst = wT3[:, ds(g0, glen), ds(ot * P, P)]
                    nc.vector.tensor_copy(dst, src)

        # bf16 view of wT: free index = 2*(icp*OSH + ot*128 + o) + h
        wTb5 = wT[:].bitcast(BF16).rearrange(
            "p (i t o h) -> p i t o h", t=O_TILES, o=P, h=2)

        # --- main loop over token tiles ---
        x_pool = ctx.enter_context(tc.tile_pool(name="xrow", bufs=3))
        tpsum_pool = ctx.enter_context(tc.tile_pool(name="tpsum", bufs=2, space="PSUM"))
        xt_pool = ctx.enter_context(tc.tile_pool(name="xt", bufs=3))
        opsum_pool = ctx.enter_context(tc.tile_pool(name="opsum", bufs=2, space="PSUM"))
        osb_pool = ctx.enter_context(tc.tile_pool(name="osb", bufs=2))

        for t in range(T_TILES):
            xh_tiles = []
            for (h0, h1) in XH:
                xt_half = x_pool.tile([P, (h1 - h0) * 256], BF16, tag="xrow")
                nc.gpsimd.dma_start(xt_half[:], x[ts(t, P), h0 * 256:h1 * 256])
                xh_tiles.append((h0, h1, xt_half))

            po = opsum_pool.tile([P, OSH], F32, tag="op")

            def x_pairs(icp):
                for (h0, h1, xt_half) in xh_tiles:
                    if h0 <= icp < h1:
                        return xt_half[:].bitcast(F32)[:, ts(icp - h0, P)]
                raise AssertionError

            emitted = []
            for gi, (g0, glen) in enumerate(groups):
                tp = tpsum_pool.tile([P, GRP * P], F32, tag="tp")
                for q in range(glen):
                    nc.tensor.transpose(
                        out=tp[:, ts(q, P)],
                        in_=x_pairs(g0 + q),
                        identity=identity[:],
                    )
                xts = xt_pool.tile([P, GRP * P], F32, tag="xt")
                nc.vector.tensor_copy(xts[:, :glen * P], tp[:, :glen * P])
                emitted.append((xts, g0, glen))
                if gi >= 1:
                    _emit_mms(nc, po, emitted[gi - 1], wTb5, IN_F // 256)
            _emit_mms(nc, po, emitted[-1], wTb5, IN_F // 256)

            osb = osb_pool.tile([P, OSH], F32, tag="osb")
            nc.vector.tensor_tensor(out=osb[:], in0=po[:], in1=scales_t[:],
                                    op=mybir.AluOpType.mult)
            nc.sync.dma_start(out[ts(t, P), :], osb[:])

    nc.compile()
    return nc


def prep_idx16(idx_shard):
    """Host prep: [OSH, NJ] int32 -> wrapped int16 gather lists
    [O_TILES*JC, 128, NJC*VDIM] matching the kernel's dma_gather layout."""
    O_TILES = OSH // P
    out = np.empty((O_TILES * JC, P, NJC * VDIM), dtype=np.int16)
    for ot in range(O_TILES):
        blk = idx_shard[ot * P:(ot + 1) * P]              # [128, NJ]
        for jc in range(JC):
            sub = blk[:, jc * NJC:(jc + 1) * NJC]          # [128, NJC]
            glist = sub.T.reshape(-1)                      # g = j*128 + o
            wrapped = glist.reshape(-1, 16).T              # [16, NJC*8]
            out[ot * JC + jc] = np.tile(wrapped, (8, 1))
    return out


_NC_CACHE = []


def _get_nc():
    if not _NC_CACHE:
        _NC_CACHE.append(build())
    return _NC_CACHE[0]


def make_in_maps(x, indices, codebook, scales):
    x2 = np.ascontiguousarray(x.reshape(T, IN_F), dtype=np.float32)
    idx2 = np.asarray(indices, dtype=np.int32).reshape(OUT_F, NJ)
    sc = np.asarray(scales, dtype=np.float32).reshape(OUT_F)
    cbv = np.ascontiguousarray(codebook, dtype=np.float32)
    in_maps = []
    for c in range(N_CORES):
        in_maps.append({
            "x": x2,
            "cb": cbv,
            "idx16": prep_idx16(idx2[c * OSH:(c + 1) * OSH]),
            "scales": np.ascontiguousarray(sc[c * OSH:(c + 1) * OSH]).reshape(1, OSH),
        })
    return in_maps


def kernel(x, indices, codebook, scales):
    nc = _get_nc()
    in_maps = make_in_maps(x, indices, codebook, scales)
    res = bass_utils.run_bass_kernel_spmd(nc, in_maps, core_ids=list(range(N_CORES)))
    out = np.concatenate([res.results[c]["out"] for c in range(N_CORES)], axis=1)
    return np.ascontiguousarray(out.reshape(BATCH, SEQ, OUT_F), dtype=np.float32)

